# revision 3
# baseline (speedup 1.0000x reference)
"""ContextGuidedTokenShift Trainium2 kernel.

Full-input contract: kernel(x=(8,16384,576) f32, weight=() f32) -> (8,16384,576) f32.

Math (per batch b, with H=W=128, token n = y*128 + xx):
    out[n, c] = w * shifted[n, c] + (1-w) * x[n, c]
    shifted[y*128+xx, c] = x[(y-dy)*128 + (xx-dx), c]   for c in slab (dy, dx),
                           0 where y-dy or xx-dx falls outside [0, 128).

Sharding: pure data-parallel over batch; core i processes x[i].

Per-core layout: a 128-token image row maps to the 128 SBUF partitions
(partition index == x coordinate), channels along the free dim.  A supertile
is R=8 consecutive image rows -> SBUF tile [128, 8*576].  The 16 channel
slabs' spatial shifts then become per-slab DMA loads from DRAM at a flat
token offset (-dy*128 - dx), with the x-wrap handled by static partition
clipping and the y-boundary by row clipping; clipped regions stay zero in a
pair of persistent staging tiles that are memset once.
"""

import numpy as np

B, H, W, C = 8, 128, 128, 576
N = H * W
R = 8                 # image rows per supertile
T = H // R            # supertiles per core
NCORES = 8

# 16 spatial offsets and their channel slab widths (cw = 64 / (|dy|+|dx|))
_OFFSETS = [(0, 1), (0, -1), (1, 0), (-1, 0), (0, 2), (0, -2), (2, 0), (-2, 0),
            (1, 1), (-1, -1), (1, -1), (-1, 1), (2, 2), (-2, -2), (2, -2), (-2, 2)]


def _build_slabs():
    slabs = []
    c = 0
    for dy, dx in _OFFSETS:
        cw = 64 // (abs(dy) + abs(dx))
        slabs.append((dy, dx, c, cw))
        c += cw
    assert c == C
    return slabs


SLABS = _build_slabs()

_CACHE = {}


def _build_bass():
    """Build the single-core Bass program (shared SPMD across all 8 cores)."""
    import concourse.bacc as bacc
    import concourse.mybir as mybir
    from concourse.tile import TileContext

    f32 = mybir.dt.float32
    nc = bacc.Bacc(
        "TRN2",
        target_bir_lowering=False,
        debug=False,
        num_devices=NCORES,
    )

    x_d = nc.dram_tensor("x", [N, C], f32, kind="ExternalInput")
    w_d = nc.dram_tensor("weight", [128, 1], f32, kind="ExternalInput")
    o_d = nc.dram_tensor("out", [N, C], f32, kind="ExternalOutput")

    x_ap = x_d.ap()
    o_ap = o_d.ap()

    FD = R * C  # free dim of a supertile

    with TileContext(nc) as tc:
        with (
            tc.tile_pool(name="const", bufs=1) as cpool,
            tc.tile_pool(name="xin", bufs=3) as xpool,
            tc.tile_pool(name="acc", bufs=3) as apool,
        ):
            # weight scalars, replicated per partition: w and (1 - w)
            w_sb = cpool.tile([128, 1], f32, tag="w")
            w1_sb = cpool.tile([128, 1], f32, tag="w1")
            nc.sync.dma_start(out=w_sb, in_=w_d.ap())
            nc.vector.tensor_scalar(
                out=w1_sb, in0=w_sb, scalar1=-1.0, scalar2=1.0,
                op0=mybir.AluOpType.mult, op1=mybir.AluOpType.add,
            )

            # persistent shifted-staging tiles (double buffered by hand so the
            # never-DMA'd boundary strips stay zero from this one-time memset)
            st_tiles = [
                cpool.tile([128, FD], f32, tag="st0", name="st0"),
                cpool.tile([128, FD], f32, tag="st1", name="st1"),
            ]
            nc.gpsimd.memset(st_tiles[0][:, :], 0.0)
            nc.gpsimd.memset(st_tiles[1][:, :], 0.0)

            for t in range(T):
                st = st_tiles[t % 2]

                # x supertile: [p, r, c] = x[(R*t + r)*128 + p, c]
                xt = xpool.tile([128, FD], f32, tag="xt")
                xt_v = xt[:, :].rearrange("p (r c) -> p r c", c=C)
                src = x_ap[R * t * W:(t + 1) * R * W, :].rearrange(
                    "(r p) c -> p r c", p=128)
                nc.sync.dma_start(out=xt_v, in_=src)

                # staging hygiene: rows that this supertile will NOT dma
                # (because the source row falls outside the image) may hold
                # stale data from supertile t-2 -> clear them.  Only the
                # last supertiles (source row >= 128, dy<0 slabs) need it;
                # t<2 slots are fresh from the initial memset.
                if t >= 2:
                    for (dy, dx, c0, cw) in SLABS:
                        r1 = min(R, H + dy - R * t)
                        if r1 < R:
                            nc.gpsimd.memset(
                                st[:, :].rearrange("p (r c) -> p r c", c=C)
                                [:, r1:R, c0:c0 + cw], 0.0)

                # per-slab shifted loads
                for (dy, dx, c0, cw) in SLABS:
                    r0 = max(0, dy - R * t)          # clip top of image
                    r1 = min(R, H + dy - R * t)      # clip bottom of image
                    if r1 <= r0:
                        continue
                    p0 = max(0, dx)
                    npart = W - abs(dx)
                    st_v = st[:, :].rearrange("p (r c) -> p r c", c=C)

                    def slab_rows(ra, rb):
                        nr = rb - ra
                        tok0 = (R * t + ra - dy) * W + p0 - dx
                        dst = st_v[p0:p0 + npart, ra:rb, c0:c0 + cw]
                        if nr == 1:
                            s = x_ap[tok0:tok0 + npart, c0:c0 + cw]
                            nc.sync.dma_start(out=dst.squeeze(1), in_=s)
                        else:
                            s = x_ap[tok0:tok0 + nr * W, c0:c0 + cw].rearrange(
                                "(r p) c -> p r c", p=128)[0:npart]
                            nc.sync.dma_start(out=dst, in_=s)

                    tok_end = (R * t + r1 - 1 - dy) * W + p0 - dx + W
                    if tok_end > N:
                        # final source row would run past the tensor end
                        # (happens for dx<0 slabs whose last source row is 127):
                        # split it into its own 2D dma
                        slab_rows(r0, r1 - 1)
                        slab_rows(r1 - 1, r1)
                    else:
                        slab_rows(r0, r1)

                # blend: out = w * shifted + (1 - w) * x
                acc = apool.tile([128, FD], f32, tag="acc")
                nc.scalar.mul(acc[:, :], xt[:, :], w1_sb[:, 0:1])
                nc.vector.scalar_tensor_tensor(
                    out=acc[:, :], in0=st[:, :], scalar=w_sb[:, 0:1],
                    in1=acc[:, :],
                    op0=mybir.AluOpType.mult, op1=mybir.AluOpType.add,
                )

                dst = o_ap[R * t * W:(t + 1) * R * W, :].rearrange(
                    "(r p) c -> p r c", p=128)
                nc.sync.dma_start(
                    out=dst, in_=acc[:, :].rearrange("p (r c) -> p r c", c=C))

    nc.compile()
    return nc


def _get_nc():
    if "nc" not in _CACHE:
        _CACHE["nc"] = _build_bass()
    return _CACHE["nc"]


def _run(x: np.ndarray, weight: np.ndarray, trace: bool = False, **kw):
    from concourse.bass_utils import run_bass_kernel_spmd

    nc = _get_nc()
    w_tile = np.full((128, 1), np.float32(weight), dtype=np.float32)
    in_maps = [
        {"x": np.ascontiguousarray(x[i], dtype=np.float32), "weight": w_tile}
        for i in range(NCORES)
    ]
    res = run_bass_kernel_spmd(
        nc, in_maps, core_ids=list(range(NCORES)), trace=trace, **kw)
    out = np.stack([r["out"] for r in res.results], axis=0)
    return out, res


def kernel(x: np.ndarray, weight: np.ndarray) -> np.ndarray:
    out, _ = _run(x, weight)
    return out


# revision 4
# speedup vs baseline: 5.5148x; 5.5148x over previous
"""ContextGuidedTokenShift Trainium2 kernel (v1: row-partition orientation).

Full-input contract: kernel(x=(8,16384,576) f32, weight=() f32) -> (8,16384,576) f32.

Math (per batch b, H=W=128, token n = y*128 + xx):
    out[n, c] = w * shifted[n, c] + (1-w) * x[n, c]
    shifted[y*128+xx, c] = x[(y-dy)*128 + (xx-dx), c]  for c in slab (dy, dx),
                           0 where y-dy or xx-dx falls outside [0, 128).

Sharding: pure data-parallel over batch; core i processes x[i].

Layout: SBUF partition p = image row y; free dim = (token-within-row, channel)
chunks of 16 tokens.  Each partition's data is contiguous in DRAM, so the
in/out DMAs are 128 x 36.9KB single-descriptor-per-partition transfers (HBM
roofline efficient; v0's orientation needed 293K tiny DMA packets and was
descriptor-generation bound at 1.46 ms).

Shifts:
  - dx (along the row) = free-dim AP offset; chunk-boundary tokens read the
    neighboring chunk's tile; x-wrap edges fall back to (1-w)*x.
  - dy (across rows) = cross-partition shift, done on the TensorEngine:
    constant 128x128 shift matrices (ones on the dy-offdiagonal, embedded in
    the NEFF) matmul the tile into PSUM; out-of-range rows come out zero.
  - diagonal slabs combine both: dy via the matrix, dx via the moving AP.

Blend: ScalarE computes t2 = (1-w)*x; VectorE scalar_tensor_tensor folds
w*shifted + t2 in one pass per region (PSUM regions for dy!=0 slabs, direct
SBUF reads for dy==0 slabs).
"""

import numpy as np

B, H, W, C = 8, 128, 128, 576
N = H * W
NCORES = 8
CHUNK = 16            # tokens per tile
NCHUNK = W // CHUNK   # 8 tiles per core
GT = 4                # tokens per PSUM group
NG = CHUNK // GT
FD = CHUNK * C        # 9216 f32 free elements per tile

# slabs: (dy, dx, c0, cw); cw = 64 // (|dy|+|dx|)
_OFFSETS = [(0, 1), (0, -1), (1, 0), (-1, 0), (0, 2), (0, -2), (2, 0), (-2, 0),
            (1, 1), (-1, -1), (1, -1), (-1, 1), (2, 2), (-2, -2), (2, -2), (-2, 2)]


def _build_slabs():
    slabs, c = [], 0
    for dy, dx in _OFFSETS:
        cw = 64 // (abs(dy) + abs(dx))
        slabs.append((dy, dx, c, cw))
        c += cw
    assert c == C
    return slabs


SLABS = _build_slabs()
A_SLABS = [s for s in SLABS if s[0] == 0]      # dy == 0: free-dim shift only
B_SLABS = [s for s in SLABS if s[0] != 0]      # dy != 0: TensorE shift
# PSUM layout (per 4-token group, 1536 f32 = 3 banks):
#   c in [128,256): idx = (c-128)*GT + t          (bank 0-1)
#   c in [320,576): idx = 512 + (c-320)*GT + t    (bank 2-5)
PSUM_FD = 1536


def _psum_off(c0):
    if 128 <= c0 < 256:
        return (c0 - 128) * GT
    assert 320 <= c0 < 576
    return 512 + (c0 - 320) * GT


def _shift_matrix(dy):
    m = np.zeros((128, 128), np.float32)
    for p in range(128):
        q = p - dy
        if 0 <= q < 128:
            m[q, p] = 1.0
    return m


_CACHE = {}


def _build_bass():
    import concourse.bacc as bacc
    import concourse.mybir as mybir
    from concourse.tile import TileContext

    f32 = mybir.dt.float32
    MULT = mybir.AluOpType.mult
    ADD = mybir.AluOpType.add

    nc = bacc.Bacc("TRN2", target_bir_lowering=False, debug=False,
                   num_devices=NCORES)

    x_d = nc.dram_tensor("x", [N, C], f32, kind="ExternalInput")
    w_d = nc.dram_tensor("weight", [128, 1], f32, kind="ExternalInput")
    o_d = nc.dram_tensor("out", [N, C], f32, kind="ExternalOutput")

    # [y, (token, channel)] views: per-partition rows are contiguous in DRAM
    x_row = x_d.ap().rearrange("(y u) c -> y (u c)", y=128)
    o_row = o_d.ap().rearrange("(y u) c -> y (u c)", y=128)

    shift_dram = {dy: nc.inline_tensor(_shift_matrix(dy), name=f"shm{dy}")
                  for dy in (1, -1, 2, -2)}

    with TileContext(nc) as tc:
        with (
            tc.tile_pool(name="const", bufs=1) as cpool,
            tc.tile_pool(name="xin", bufs=3) as xpool,
            tc.tile_pool(name="acc", bufs=2) as apool,
            tc.tile_pool(name="ps", bufs=2, space="PSUM") as pspool,
        ):
            w_sb = cpool.tile([128, 1], f32, tag="w", name="w_sb")
            w1_sb = cpool.tile([128, 1], f32, tag="w1", name="w1_sb")
            nc.sync.dma_start(out=w_sb, in_=w_d.ap())
            nc.vector.tensor_scalar(out=w1_sb, in0=w_sb, scalar1=-1.0,
                                    scalar2=1.0, op0=MULT, op1=ADD)

            smat = {}
            for dy in (1, -1, 2, -2):
                smat[dy] = cpool.tile([128, 128], f32, tag=f"sm{dy}",
                                      name=f"sm{dy}")
                nc.sync.dma_start(out=smat[dy], in_=shift_dram[dy].ap())

            zt = cpool.tile([128, 256], f32, tag="zt", name="zt")
            nc.gpsimd.memset(zt, 0.0)

            def zmov(cw, ec):
                # arbitrary zero-valued moving operand of shape (cw, ec)
                return zt.rearrange("p (a b) -> p a b", b=ec)[:, 0:cw, :]

            xts = {}

            def mm(ps, po, tlo, thi, dy, src3, s_tok, c0, cw):
                """psum[:, (c: cw @po stride GT), (t: tlo..thi)] =
                   S_dy.T @ src3[:, s_tok.., c0:c0+cw] (moving dims (c, t))."""
                out = ps.rearrange("p (c t) -> p c t", t=GT)[
                    :, po // GT:po // GT + cw, tlo:thi]
                if src3 is None:
                    mov = zmov(cw, thi - tlo)
                else:
                    mov = src3[:, s_tok:s_tok + (thi - tlo),
                               c0:c0 + cw].transpose([0, 2, 1])
                nc.tensor.matmul(out, smat[dy], mov, start=True, stop=True)

            def compute(k):
                xt = xts[k]
                xt3 = xt.rearrange("p (t c) -> p t c", c=C)
                prev3 = (xts[k - 1].rearrange("p (t c) -> p t c", c=C)
                         if k > 0 else None)
                next3 = (xts[k + 1].rearrange("p (t c) -> p t c", c=C)
                         if k < NCHUNK - 1 else None)

                t2 = apool.tile([128, FD], f32, tag="t2", name="t2")
                t2v = t2.rearrange("p (t c) -> p t c", c=C)
                nc.scalar.mul(t2, xt, w1_sb[:, 0:1])

                for g in range(NG):
                    t0 = g * GT
                    ps = pspool.tile([128, PSUM_FD], f32, tag="ps", name="ps")
                    for (dy, dx, c0, cw) in B_SLABS:
                        po = _psum_off(c0)
                        if dx == 0:
                            mm(ps, po, 0, GT, dy, xt3, t0, c0, cw)
                            continue
                        # token i (in group) sources chunk-token t0+i-dx
                        lo = max(0, dx - t0)               # from prev chunk
                        hi = min(GT, CHUNK + dx - t0)      # below: next chunk
                        if hi > lo:
                            mm(ps, po, lo, hi, dy, xt3, t0 + lo - dx, c0, cw)
                        if lo > 0:
                            mm(ps, po, 0, lo, dy, prev3,
                               CHUNK + t0 - dx, c0, cw)
                        if hi < GT:
                            mm(ps, po, hi, GT, dy, next3,
                               t0 + hi - dx - CHUNK, c0, cw)
                    # blend psum regions: out = w*psum + t2  (in place)
                    for (coff, clen, poff) in ((128, 128, 0), (320, 256, 512)):
                        dst = t2v[:, t0:t0 + GT, coff:coff + clen]
                        src = ps.rearrange("p (c t) -> p t c", t=GT)[
                            :, 0:GT, poff // GT:poff // GT + clen]
                        nc.vector.scalar_tensor_tensor(
                            out=dst, in0=src, scalar=w_sb[:, 0:1], in1=dst,
                            op0=MULT, op1=ADD)

                # dy==0 slabs: free-dim shifted SBUF reads
                for (dy, dx, c0, cw) in A_SLABS:
                    lo = max(0, dx)
                    hi = CHUNK + min(0, dx)
                    dst = t2v[:, lo:hi, c0:c0 + cw]
                    src = xt3[:, lo - dx:hi - dx, c0:c0 + cw]
                    nc.vector.scalar_tensor_tensor(
                        out=dst, in0=src, scalar=w_sb[:, 0:1], in1=dst,
                        op0=MULT, op1=ADD)
                    if dx > 0 and prev3 is not None:
                        dst = t2v[:, 0:dx, c0:c0 + cw]
                        src = prev3[:, CHUNK - dx:CHUNK, c0:c0 + cw]
                        nc.vector.scalar_tensor_tensor(
                            out=dst, in0=src, scalar=w_sb[:, 0:1], in1=dst,
                            op0=MULT, op1=ADD)
                    elif dx < 0 and next3 is not None:
                        dst = t2v[:, CHUNK + dx:CHUNK, c0:c0 + cw]
                        src = next3[:, 0:-dx, c0:c0 + cw]
                        nc.vector.scalar_tensor_tensor(
                            out=dst, in0=src, scalar=w_sb[:, 0:1], in1=dst,
                            op0=MULT, op1=ADD)
                    # x-wrap edge (k==0 for dx>0, k==NCHUNK-1 for dx<0):
                    # shifted is 0 there, t2 already holds (1-w)*x -> no op.

                nc.scalar.dma_start(out=o_row[:, k * FD:(k + 1) * FD], in_=t2)

            for k in range(NCHUNK):
                xts[k] = xpool.tile([128, FD], f32, tag="xt", name="xt")
                nc.sync.dma_start(out=xts[k], in_=x_row[:, k * FD:(k + 1) * FD])
                if k >= 1:
                    compute(k - 1)
            compute(NCHUNK - 1)

    nc.compile()
    return nc


def _get_nc():
    if "nc" not in _CACHE:
        _CACHE["nc"] = _build_bass()
    return _CACHE["nc"]


def _run(x: np.ndarray, weight: np.ndarray, trace: bool = False, **kw):
    from concourse.bass_utils import run_bass_kernel_spmd

    nc = _get_nc()
    w_tile = np.full((128, 1), np.float32(weight), dtype=np.float32)
    in_maps = [
        {"x": np.ascontiguousarray(x[i], dtype=np.float32), "weight": w_tile}
        for i in range(NCORES)
    ]
    res = run_bass_kernel_spmd(
        nc, in_maps, core_ids=list(range(NCORES)), trace=trace, **kw)
    out = np.stack([r["out"] for r in res.results], axis=0)
    return out, res


def kernel(x: np.ndarray, weight: np.ndarray) -> np.ndarray:
    out, _ = _run(x, weight)
    return out


# revision 10
# speedup vs baseline: 5.7854x; 1.0491x over previous
"""ContextGuidedTokenShift Trainium2 kernel (v1: row-partition orientation).

Full-input contract: kernel(x=(8,16384,576) f32, weight=() f32) -> (8,16384,576) f32.

Math (per batch b, H=W=128, token n = y*128 + xx):
    out[n, c] = w * shifted[n, c] + (1-w) * x[n, c]
    shifted[y*128+xx, c] = x[(y-dy)*128 + (xx-dx), c]  for c in slab (dy, dx),
                           0 where y-dy or xx-dx falls outside [0, 128).

Sharding: pure data-parallel over batch; core i processes x[i].

Layout: SBUF partition p = image row y; free dim = (token-within-row, channel)
chunks of 16 tokens.  Each partition's data is contiguous in DRAM, so the
in/out DMAs are 128 x 36.9KB single-descriptor-per-partition transfers (HBM
roofline efficient; v0's orientation needed 293K tiny DMA packets and was
descriptor-generation bound at 1.46 ms).

Shifts:
  - dx (along the row) = free-dim AP offset; chunk-boundary tokens read the
    neighboring chunk's tile; x-wrap edges fall back to (1-w)*x.
  - dy (across rows) = cross-partition shift, done on the TensorEngine:
    constant 128x128 shift matrices (ones on the dy-offdiagonal, embedded in
    the NEFF) matmul the tile into PSUM; out-of-range rows come out zero.
  - diagonal slabs combine both: dy via the matrix, dx via the moving AP.

Blend: ScalarE computes t2 = (1-w)*x; VectorE scalar_tensor_tensor folds
w*shifted + t2 in one pass per region (PSUM regions for dy!=0 slabs, direct
SBUF reads for dy==0 slabs).
"""

import numpy as np

B, H, W, C = 8, 128, 128, 576
N = H * W
NCORES = 8
CHUNK = 16            # tokens per tile
NCHUNK = W // CHUNK   # 8 tiles per core
GT = 8                # tokens per PSUM group
NG = CHUNK // GT
FD = CHUNK * C        # 9216 f32 free elements per tile

# slabs: (dy, dx, c0, cw); cw = 64 // (|dy|+|dx|)
_OFFSETS = [(0, 1), (0, -1), (1, 0), (-1, 0), (0, 2), (0, -2), (2, 0), (-2, 0),
            (1, 1), (-1, -1), (1, -1), (-1, 1), (2, 2), (-2, -2), (2, -2), (-2, 2)]


def _build_slabs():
    slabs, c = [], 0
    for dy, dx in _OFFSETS:
        cw = 64 // (abs(dy) + abs(dx))
        slabs.append((dy, dx, c, cw))
        c += cw
    assert c == C
    return slabs


SLABS = _build_slabs()
A_SLABS = [s for s in SLABS if s[0] == 0]      # dy == 0: free-dim shift only
B_SLABS = [s for s in SLABS if s[0] != 0]      # dy != 0: TensorE shift
# Two PSUM regions per 8-token group (split pools so B1 double-buffers
# within the 8-bank budget: B1 2 banks x2 bufs + B2 4 banks x1 buf):
#   B1: c in [128,256): idx = (c-128)*GT + t   (1024 f32, 2 banks)
#   B2: c in [320,576): idx = (c-320)*GT + t   (2048 f32, 4 banks)
B1_FD, B2_FD = 1024, 2048


def _psum_region(c0):
    if 128 <= c0 < 256:
        return "b1", (c0 - 128) * GT
    assert 320 <= c0 < 576
    return "b2", (c0 - 320) * GT


def _shift_matrix(dy):
    m = np.zeros((128, 128), np.float32)
    for p in range(128):
        q = p - dy
        if 0 <= q < 128:
            m[q, p] = 1.0
    return m


_CACHE = {}


def _build_bass():
    import concourse.bacc as bacc
    import concourse.mybir as mybir
    from concourse.tile import TileContext

    f32 = mybir.dt.float32
    MULT = mybir.AluOpType.mult
    ADD = mybir.AluOpType.add

    nc = bacc.Bacc("TRN2", target_bir_lowering=False, debug=False,
                   num_devices=NCORES)

    x_d = nc.dram_tensor("x", [N, C], f32, kind="ExternalInput")
    w_d = nc.dram_tensor("weight", [128, 1], f32, kind="ExternalInput")
    o_d = nc.dram_tensor("out", [N, C], f32, kind="ExternalOutput")

    # [y, (token, channel)] views: per-partition rows are contiguous in DRAM
    x_row = x_d.ap().rearrange("(y u) c -> y (u c)", y=128)
    o_row = o_d.ap().rearrange("(y u) c -> y (u c)", y=128)

    shift_dram = {dy: nc.inline_tensor(_shift_matrix(dy), name=f"shm{dy}")
                  for dy in (1, -1, 2, -2)}

    with TileContext(nc) as tc:
        with (
            tc.tile_pool(name="const", bufs=1) as cpool,
            tc.tile_pool(name="xin", bufs=3) as xpool,
            tc.tile_pool(name="acc", bufs=2) as apool,
            tc.tile_pool(name="psb1", bufs=2, space="PSUM") as psb1pool,
            tc.tile_pool(name="psb2", bufs=1, space="PSUM") as psb2pool,
        ):
            w_sb = cpool.tile([128, 1], f32, tag="w", name="w_sb")
            w1_sb = cpool.tile([128, 1], f32, tag="w1", name="w1_sb")
            nc.sync.dma_start(out=w_sb, in_=w_d.ap())
            nc.vector.tensor_scalar(out=w1_sb, in0=w_sb, scalar1=-1.0,
                                    scalar2=1.0, op0=MULT, op1=ADD)

            smat = {}
            for dy in (1, -1, 2, -2):
                smat[dy] = cpool.tile([128, 128], f32, tag=f"sm{dy}",
                                      name=f"sm{dy}")
                nc.sync.dma_start(out=smat[dy], in_=shift_dram[dy].ap())

            zt = cpool.tile([128, 256], f32, tag="zt", name="zt")
            nc.gpsimd.memset(zt, 0.0)

            def zmov(cw, ec):
                # arbitrary zero-valued moving operand of shape (cw, ec)
                return zt.rearrange("p (a b) -> p a b", b=ec)[:, 0:cw, :]

            xts = {}

            def mm(ps, po, tlo, thi, dy, src3, s_tok, c0, cw):
                """psum[:, (c: cw @po stride GT), (t: tlo..thi)] =
                   S_dy.T @ src3[:, s_tok.., c0:c0+cw] (moving dims (c, t))."""
                out = ps.rearrange("p (c t) -> p c t", t=GT)[
                    :, po // GT:po // GT + cw, tlo:thi]
                if src3 is None:
                    mov = zmov(cw, thi - tlo)
                else:
                    mov = src3[:, s_tok:s_tok + (thi - tlo),
                               c0:c0 + cw].transpose([0, 2, 1])
                nc.tensor.matmul(out, smat[dy], mov, start=True, stop=True)

            def compute(k):
                xt = xts[k]
                xt3 = xt.rearrange("p (t c) -> p t c", c=C)
                prev3 = (xts[k - 1].rearrange("p (t c) -> p t c", c=C)
                         if k > 0 else None)
                next3 = (xts[k + 1].rearrange("p (t c) -> p t c", c=C)
                         if k < NCHUNK - 1 else None)

                t2 = apool.tile([128, FD], f32, tag="t2", name="t2")
                t2v = t2.rearrange("p (t c) -> p t c", c=C)
                nc.scalar.mul(t2, xt, w1_sb[:, 0:1])

                for g in range(NG):
                    t0 = g * GT
                    psb1 = psb1pool.tile([128, B1_FD], f32, tag="b1", name="b1")
                    psb2 = psb2pool.tile([128, B2_FD], f32, tag="b2", name="b2")
                    regions = {"b1": psb1, "b2": psb2}
                    for (dy, dx, c0, cw) in B_SLABS:
                        reg, po = _psum_region(c0)
                        ps = regions[reg]
                        if dx == 0:
                            mm(ps, po, 0, GT, dy, xt3, t0, c0, cw)
                            continue
                        # token i (in group) sources chunk-token t0+i-dx
                        lo = max(0, dx - t0)               # from prev chunk
                        hi = min(GT, CHUNK + dx - t0)      # below: next chunk
                        if hi > lo:
                            mm(ps, po, lo, hi, dy, xt3, t0 + lo - dx, c0, cw)
                        if lo > 0:
                            mm(ps, po, 0, lo, dy, prev3,
                               CHUNK + t0 - dx, c0, cw)
                        if hi < GT:
                            mm(ps, po, hi, GT, dy, next3,
                               t0 + hi - dx - CHUNK, c0, cw)
                    # blend psum regions: out = w*psum + t2  (in place)
                    for (coff, clen, ps) in ((128, 128, psb1), (320, 256, psb2)):
                        dst = t2v[:, t0:t0 + GT, coff:coff + clen]
                        src = ps.rearrange("p (c t) -> p t c", t=GT)[
                            :, 0:GT, 0:clen]
                        nc.vector.scalar_tensor_tensor(
                            out=dst, in0=src, scalar=w_sb[:, 0:1], in1=dst,
                            op0=MULT, op1=ADD)

                # dy==0 slabs: free-dim shifted SBUF reads
                for (dy, dx, c0, cw) in A_SLABS:
                    lo = max(0, dx)
                    hi = CHUNK + min(0, dx)
                    dst = t2v[:, lo:hi, c0:c0 + cw]
                    src = xt3[:, lo - dx:hi - dx, c0:c0 + cw]
                    nc.vector.scalar_tensor_tensor(
                        out=dst, in0=src, scalar=w_sb[:, 0:1], in1=dst,
                        op0=MULT, op1=ADD)
                    if dx > 0 and prev3 is not None:
                        dst = t2v[:, 0:dx, c0:c0 + cw]
                        src = prev3[:, CHUNK - dx:CHUNK, c0:c0 + cw]
                        nc.vector.scalar_tensor_tensor(
                            out=dst, in0=src, scalar=w_sb[:, 0:1], in1=dst,
                            op0=MULT, op1=ADD)
                    elif dx < 0 and next3 is not None:
                        dst = t2v[:, CHUNK + dx:CHUNK, c0:c0 + cw]
                        src = next3[:, 0:-dx, c0:c0 + cw]
                        nc.vector.scalar_tensor_tensor(
                            out=dst, in0=src, scalar=w_sb[:, 0:1], in1=dst,
                            op0=MULT, op1=ADD)
                    # x-wrap edge (k==0 for dx>0, k==NCHUNK-1 for dx<0):
                    # shifted is 0 there, t2 already holds (1-w)*x -> no op.

                nc.scalar.dma_start(out=o_row[:, k * FD:(k + 1) * FD], in_=t2)

            for k in range(NCHUNK):
                xts[k] = xpool.tile([128, FD], f32, tag="xt", name="xt")
                nc.sync.dma_start(out=xts[k], in_=x_row[:, k * FD:(k + 1) * FD])
                if k >= 1:
                    compute(k - 1)
            compute(NCHUNK - 1)

    nc.compile()
    return nc


def _get_nc():
    if "nc" not in _CACHE:
        _CACHE["nc"] = _build_bass()
    return _CACHE["nc"]


def _run(x: np.ndarray, weight: np.ndarray, trace: bool = False, **kw):
    from concourse.bass_utils import run_bass_kernel_spmd

    nc = _get_nc()
    w_tile = np.full((128, 1), np.float32(weight), dtype=np.float32)
    in_maps = [
        {"x": np.ascontiguousarray(x[i], dtype=np.float32), "weight": w_tile}
        for i in range(NCORES)
    ]
    res = run_bass_kernel_spmd(
        nc, in_maps, core_ids=list(range(NCORES)), trace=trace, **kw)
    out = np.stack([r["out"] for r in res.results], axis=0)
    return out, res


def kernel(x: np.ndarray, weight: np.ndarray) -> np.ndarray:
    out, _ = _run(x, weight)
    return out


# revision 12
# speedup vs baseline: 6.3303x; 1.0942x over previous
"""ContextGuidedTokenShift Trainium2 kernel (v1: row-partition orientation).

Full-input contract: kernel(x=(8,16384,576) f32, weight=() f32) -> (8,16384,576) f32.

Math (per batch b, H=W=128, token n = y*128 + xx):
    out[n, c] = w * shifted[n, c] + (1-w) * x[n, c]
    shifted[y*128+xx, c] = x[(y-dy)*128 + (xx-dx), c]  for c in slab (dy, dx),
                           0 where y-dy or xx-dx falls outside [0, 128).

Sharding: pure data-parallel over batch; core i processes x[i].

Layout: SBUF partition p = image row y; free dim = (token-within-row, channel)
chunks of 16 tokens.  Each partition's data is contiguous in DRAM, so the
in/out DMAs are 128 x 36.9KB single-descriptor-per-partition transfers (HBM
roofline efficient; v0's orientation needed 293K tiny DMA packets and was
descriptor-generation bound at 1.46 ms).

Shifts:
  - dx (along the row) = free-dim AP offset; chunk-boundary tokens read the
    neighboring chunk's tile; x-wrap edges fall back to (1-w)*x.
  - dy (across rows) = cross-partition shift, done on the TensorEngine:
    constant 128x128 shift matrices (ones on the dy-offdiagonal, embedded in
    the NEFF) matmul the tile into PSUM; out-of-range rows come out zero.
  - diagonal slabs combine both: dy via the matrix, dx via the moving AP.

Blend: ScalarE computes t2 = (1-w)*x; VectorE scalar_tensor_tensor folds
w*shifted + t2 in one pass per region (PSUM regions for dy!=0 slabs, direct
SBUF reads for dy==0 slabs).
"""

import numpy as np

B, H, W, C = 8, 128, 128, 576
N = H * W
NCORES = 8
CHUNK = 8             # tokens per tile
NCHUNK = W // CHUNK   # tiles per core
GT = 8                # tokens per PSUM group
NG = CHUNK // GT
FD = CHUNK * C        # f32 free elements per tile

# slabs: (dy, dx, c0, cw); cw = 64 // (|dy|+|dx|)
_OFFSETS = [(0, 1), (0, -1), (1, 0), (-1, 0), (0, 2), (0, -2), (2, 0), (-2, 0),
            (1, 1), (-1, -1), (1, -1), (-1, 1), (2, 2), (-2, -2), (2, -2), (-2, 2)]


def _build_slabs():
    slabs, c = [], 0
    for dy, dx in _OFFSETS:
        cw = 64 // (abs(dy) + abs(dx))
        slabs.append((dy, dx, c, cw))
        c += cw
    assert c == C
    return slabs


SLABS = _build_slabs()
A_SLABS = [s for s in SLABS if s[0] == 0]      # dy == 0: free-dim shift only
B_SLABS = [s for s in SLABS if s[0] != 0]      # dy != 0: TensorE shift
# Two PSUM regions per 8-token group (split pools so B1 double-buffers
# within the 8-bank budget: B1 2 banks x2 bufs + B2 4 banks x1 buf):
#   B1: c in [128,256): idx = (c-128)*GT + t   (1024 f32, 2 banks)
#   B2: c in [320,576): idx = (c-320)*GT + t   (2048 f32, 4 banks)
B1_FD, B2_FD = 1024, 2048


def _psum_region(c0):
    if 128 <= c0 < 256:
        return "b1", (c0 - 128) * GT
    assert 320 <= c0 < 576
    return "b2", (c0 - 320) * GT


def _shift_matrix(dy):
    m = np.zeros((128, 128), np.float32)
    for p in range(128):
        q = p - dy
        if 0 <= q < 128:
            m[q, p] = 1.0
    return m


_CACHE = {}


def _build_bass():
    import concourse.bacc as bacc
    import concourse.mybir as mybir
    from concourse.tile import TileContext

    f32 = mybir.dt.float32
    MULT = mybir.AluOpType.mult
    ADD = mybir.AluOpType.add

    nc = bacc.Bacc("TRN2", target_bir_lowering=False, debug=False,
                   num_devices=NCORES)

    x_d = nc.dram_tensor("x", [N, C], f32, kind="ExternalInput")
    w_d = nc.dram_tensor("weight", [128, 1], f32, kind="ExternalInput")
    o_d = nc.dram_tensor("out", [N, C], f32, kind="ExternalOutput")

    # [y, (token, channel)] views: per-partition rows are contiguous in DRAM
    x_row = x_d.ap().rearrange("(y u) c -> y (u c)", y=128)
    o_row = o_d.ap().rearrange("(y u) c -> y (u c)", y=128)

    shift_dram = {dy: nc.inline_tensor(_shift_matrix(dy), name=f"shm{dy}")
                  for dy in (1, -1, 2, -2)}

    with TileContext(nc) as tc:
        with (
            tc.tile_pool(name="const", bufs=1) as cpool,
            tc.tile_pool(name="xin", bufs=4) as xpool,
            tc.tile_pool(name="acc", bufs=3) as apool,
            tc.tile_pool(name="psb1", bufs=2, space="PSUM") as psb1pool,
            tc.tile_pool(name="psb2", bufs=1, space="PSUM") as psb2pool,
        ):
            w_sb = cpool.tile([128, 1], f32, tag="w", name="w_sb")
            w1_sb = cpool.tile([128, 1], f32, tag="w1", name="w1_sb")
            nc.sync.dma_start(out=w_sb, in_=w_d.ap())
            nc.vector.tensor_scalar(out=w1_sb, in0=w_sb, scalar1=-1.0,
                                    scalar2=1.0, op0=MULT, op1=ADD)

            smat = {}
            for dy in (1, -1, 2, -2):
                smat[dy] = cpool.tile([128, 128], f32, tag=f"sm{dy}",
                                      name=f"sm{dy}")
                nc.sync.dma_start(out=smat[dy], in_=shift_dram[dy].ap())

            zt = cpool.tile([128, 256], f32, tag="zt", name="zt")
            nc.gpsimd.memset(zt, 0.0)

            def zmov(cw, ec):
                # arbitrary zero-valued moving operand of shape (cw, ec)
                return zt.rearrange("p (a b) -> p a b", b=ec)[:, 0:cw, :]

            xts = {}

            def mm(ps, po, tlo, thi, dy, src3, s_tok, c0, cw):
                """psum[:, (c: cw @po stride GT), (t: tlo..thi)] =
                   S_dy.T @ src3[:, s_tok.., c0:c0+cw] (moving dims (c, t))."""
                out = ps.rearrange("p (c t) -> p c t", t=GT)[
                    :, po // GT:po // GT + cw, tlo:thi]
                if src3 is None:
                    mov = zmov(cw, thi - tlo)
                else:
                    mov = src3[:, s_tok:s_tok + (thi - tlo),
                               c0:c0 + cw].transpose([0, 2, 1])
                nc.tensor.matmul(out, smat[dy], mov, start=True, stop=True)

            def compute(k):
                xt = xts[k]
                xt3 = xt.rearrange("p (t c) -> p t c", c=C)
                prev3 = (xts[k - 1].rearrange("p (t c) -> p t c", c=C)
                         if k > 0 else None)
                next3 = (xts[k + 1].rearrange("p (t c) -> p t c", c=C)
                         if k < NCHUNK - 1 else None)

                t2 = apool.tile([128, FD], f32, tag="t2", name="t2")
                t2v = t2.rearrange("p (t c) -> p t c", c=C)
                nc.scalar.mul(t2, xt, w1_sb[:, 0:1])

                for g in range(NG):
                    t0 = g * GT
                    psb1 = psb1pool.tile([128, B1_FD], f32, tag="b1", name="b1")
                    psb2 = psb2pool.tile([128, B2_FD], f32, tag="b2", name="b2")
                    regions = {"b1": psb1, "b2": psb2}
                    for (dy, dx, c0, cw) in B_SLABS:
                        reg, po = _psum_region(c0)
                        ps = regions[reg]
                        if dx == 0:
                            mm(ps, po, 0, GT, dy, xt3, t0, c0, cw)
                            continue
                        # token i (in group) sources chunk-token t0+i-dx
                        lo = max(0, dx - t0)               # from prev chunk
                        hi = min(GT, CHUNK + dx - t0)      # below: next chunk
                        if hi > lo:
                            mm(ps, po, lo, hi, dy, xt3, t0 + lo - dx, c0, cw)
                        if lo > 0:
                            mm(ps, po, 0, lo, dy, prev3,
                               CHUNK + t0 - dx, c0, cw)
                        if hi < GT:
                            mm(ps, po, hi, GT, dy, next3,
                               t0 + hi - dx - CHUNK, c0, cw)
                    # blend psum regions: out = w*psum + t2  (in place)
                    for (coff, clen, ps) in ((128, 128, psb1), (320, 256, psb2)):
                        dst = t2v[:, t0:t0 + GT, coff:coff + clen]
                        src = ps.rearrange("p (c t) -> p t c", t=GT)[
                            :, 0:GT, 0:clen]
                        nc.vector.scalar_tensor_tensor(
                            out=dst, in0=src, scalar=w_sb[:, 0:1], in1=dst,
                            op0=MULT, op1=ADD)

                # dy==0 slabs: free-dim shifted SBUF reads
                for (dy, dx, c0, cw) in A_SLABS:
                    lo = max(0, dx)
                    hi = CHUNK + min(0, dx)
                    dst = t2v[:, lo:hi, c0:c0 + cw]
                    src = xt3[:, lo - dx:hi - dx, c0:c0 + cw]
                    nc.vector.scalar_tensor_tensor(
                        out=dst, in0=src, scalar=w_sb[:, 0:1], in1=dst,
                        op0=MULT, op1=ADD)
                    if dx > 0 and prev3 is not None:
                        dst = t2v[:, 0:dx, c0:c0 + cw]
                        src = prev3[:, CHUNK - dx:CHUNK, c0:c0 + cw]
                        nc.vector.scalar_tensor_tensor(
                            out=dst, in0=src, scalar=w_sb[:, 0:1], in1=dst,
                            op0=MULT, op1=ADD)
                    elif dx < 0 and next3 is not None:
                        dst = t2v[:, CHUNK + dx:CHUNK, c0:c0 + cw]
                        src = next3[:, 0:-dx, c0:c0 + cw]
                        nc.vector.scalar_tensor_tensor(
                            out=dst, in0=src, scalar=w_sb[:, 0:1], in1=dst,
                            op0=MULT, op1=ADD)
                    # x-wrap edge (k==0 for dx>0, k==NCHUNK-1 for dx<0):
                    # shifted is 0 there, t2 already holds (1-w)*x -> no op.

                nc.scalar.dma_start(out=o_row[:, k * FD:(k + 1) * FD], in_=t2)

            for k in range(NCHUNK):
                xts[k] = xpool.tile([128, FD], f32, tag="xt", name="xt")
                nc.sync.dma_start(out=xts[k], in_=x_row[:, k * FD:(k + 1) * FD])
                if k >= 1:
                    compute(k - 1)
            compute(NCHUNK - 1)

    nc.compile()
    return nc


def _get_nc():
    if "nc" not in _CACHE:
        _CACHE["nc"] = _build_bass()
    return _CACHE["nc"]


def _run(x: np.ndarray, weight: np.ndarray, trace: bool = False, **kw):
    from concourse.bass_utils import run_bass_kernel_spmd

    nc = _get_nc()
    w_tile = np.full((128, 1), np.float32(weight), dtype=np.float32)
    in_maps = [
        {"x": np.ascontiguousarray(x[i], dtype=np.float32), "weight": w_tile}
        for i in range(NCORES)
    ]
    res = run_bass_kernel_spmd(
        nc, in_maps, core_ids=list(range(NCORES)), trace=trace, **kw)
    out = np.stack([r["out"] for r in res.results], axis=0)
    return out, res


def kernel(x: np.ndarray, weight: np.ndarray) -> np.ndarray:
    out, _ = _run(x, weight)
    return out


# revision 13
# speedup vs baseline: 6.4139x; 1.0132x over previous
"""ContextGuidedTokenShift Trainium2 kernel (v1: row-partition orientation).

Full-input contract: kernel(x=(8,16384,576) f32, weight=() f32) -> (8,16384,576) f32.

Math (per batch b, H=W=128, token n = y*128 + xx):
    out[n, c] = w * shifted[n, c] + (1-w) * x[n, c]
    shifted[y*128+xx, c] = x[(y-dy)*128 + (xx-dx), c]  for c in slab (dy, dx),
                           0 where y-dy or xx-dx falls outside [0, 128).

Sharding: pure data-parallel over batch; core i processes x[i].

Layout: SBUF partition p = image row y; free dim = (token-within-row, channel)
chunks of 16 tokens.  Each partition's data is contiguous in DRAM, so the
in/out DMAs are 128 x 36.9KB single-descriptor-per-partition transfers (HBM
roofline efficient; v0's orientation needed 293K tiny DMA packets and was
descriptor-generation bound at 1.46 ms).

Shifts:
  - dx (along the row) = free-dim AP offset; chunk-boundary tokens read the
    neighboring chunk's tile; x-wrap edges fall back to (1-w)*x.
  - dy (across rows) = cross-partition shift, done on the TensorEngine:
    constant 128x128 shift matrices (ones on the dy-offdiagonal, embedded in
    the NEFF) matmul the tile into PSUM; out-of-range rows come out zero.
  - diagonal slabs combine both: dy via the matrix, dx via the moving AP.

Blend: ScalarE computes t2 = (1-w)*x; VectorE scalar_tensor_tensor folds
w*shifted + t2 in one pass per region (PSUM regions for dy!=0 slabs, direct
SBUF reads for dy==0 slabs).
"""

import numpy as np

B, H, W, C = 8, 128, 128, 576
N = H * W
NCORES = 8
CHUNK = 16            # tokens per tile
NCHUNK = W // CHUNK   # 8 tiles per core
GT = 8                # tokens per PSUM group
NG = CHUNK // GT
FD = CHUNK * C        # 9216 f32 free elements per tile

# slabs: (dy, dx, c0, cw); cw = 64 // (|dy|+|dx|)
_OFFSETS = [(0, 1), (0, -1), (1, 0), (-1, 0), (0, 2), (0, -2), (2, 0), (-2, 0),
            (1, 1), (-1, -1), (1, -1), (-1, 1), (2, 2), (-2, -2), (2, -2), (-2, 2)]


def _build_slabs():
    slabs, c = [], 0
    for dy, dx in _OFFSETS:
        cw = 64 // (abs(dy) + abs(dx))
        slabs.append((dy, dx, c, cw))
        c += cw
    assert c == C
    return slabs


SLABS = _build_slabs()
A_SLABS = [s for s in SLABS if s[0] == 0]      # dy == 0: free-dim shift only
B_SLABS = [s for s in SLABS if s[0] != 0]      # dy != 0: TensorE shift
# Two PSUM regions per 8-token group (split pools so B1 double-buffers
# within the 8-bank budget: B1 2 banks x2 bufs + B2 4 banks x1 buf):
#   B1: c in [128,256): idx = (c-128)*GT + t   (1024 f32, 2 banks)
#   B2: c in [320,576): idx = (c-320)*GT + t   (2048 f32, 4 banks)
B1_FD, B2_FD = 1024, 2048


def _psum_region(c0):
    if 128 <= c0 < 256:
        return "b1", (c0 - 128) * GT
    assert 320 <= c0 < 576
    return "b2", (c0 - 320) * GT


def _shift_matrix(dy):
    m = np.zeros((128, 128), np.float32)
    for p in range(128):
        q = p - dy
        if 0 <= q < 128:
            m[q, p] = 1.0
    return m


_CACHE = {}


def _build_bass():
    import concourse.bacc as bacc
    import concourse.mybir as mybir
    from concourse.tile import TileContext

    f32 = mybir.dt.float32
    MULT = mybir.AluOpType.mult
    ADD = mybir.AluOpType.add

    nc = bacc.Bacc("TRN2", target_bir_lowering=False, debug=False,
                   num_devices=NCORES)

    x_d = nc.dram_tensor("x", [N, C], f32, kind="ExternalInput")
    w_d = nc.dram_tensor("weight", [128, 1], f32, kind="ExternalInput")
    o_d = nc.dram_tensor("out", [N, C], f32, kind="ExternalOutput")

    # [y, (token, channel)] views: per-partition rows are contiguous in DRAM
    x_row = x_d.ap().rearrange("(y u) c -> y (u c)", y=128)
    o_row = o_d.ap().rearrange("(y u) c -> y (u c)", y=128)

    shift_dram = {dy: nc.inline_tensor(_shift_matrix(dy), name=f"shm{dy}")
                  for dy in (1, -1, 2, -2)}

    with TileContext(nc) as tc:
        with (
            tc.tile_pool(name="const", bufs=1) as cpool,
            tc.tile_pool(name="xin", bufs=3) as xpool,
            tc.tile_pool(name="acc", bufs=2) as apool,
            tc.tile_pool(name="psb1", bufs=2, space="PSUM") as psb1pool,
            tc.tile_pool(name="psb2", bufs=1, space="PSUM") as psb2pool,
        ):
            w_sb = cpool.tile([128, 1], f32, tag="w", name="w_sb")
            w1_sb = cpool.tile([128, 1], f32, tag="w1", name="w1_sb")
            nc.sync.dma_start(out=w_sb, in_=w_d.ap())
            nc.vector.tensor_scalar(out=w1_sb, in0=w_sb, scalar1=-1.0,
                                    scalar2=1.0, op0=MULT, op1=ADD)

            smat = {}
            for dy in (1, -1, 2, -2):
                smat[dy] = cpool.tile([128, 128], f32, tag=f"sm{dy}",
                                      name=f"sm{dy}")
                nc.sync.dma_start(out=smat[dy], in_=shift_dram[dy].ap())

            zt = cpool.tile([128, 256], f32, tag="zt", name="zt")
            nc.gpsimd.memset(zt, 0.0)

            def zmov(cw, ec):
                # arbitrary zero-valued moving operand of shape (cw, ec)
                return zt.rearrange("p (a b) -> p a b", b=ec)[:, 0:cw, :]

            xts = {}

            def mm(ps, po, tlo, thi, dy, src3, s_tok, c0, cw):
                """psum[:, (c: cw @po stride GT), (t: tlo..thi)] =
                   S_dy.T @ src3[:, s_tok.., c0:c0+cw] (moving dims (c, t))."""
                out = ps.rearrange("p (c t) -> p c t", t=GT)[
                    :, po // GT:po // GT + cw, tlo:thi]
                if src3 is None:
                    mov = zmov(cw, thi - tlo)
                else:
                    mov = src3[:, s_tok:s_tok + (thi - tlo),
                               c0:c0 + cw].transpose([0, 2, 1])
                nc.tensor.matmul(out, smat[dy], mov, start=True, stop=True)

            def compute(k):
                xt = xts[k]
                xt3 = xt.rearrange("p (t c) -> p t c", c=C)
                prev3 = (xts[k - 1].rearrange("p (t c) -> p t c", c=C)
                         if k > 0 else None)
                next3 = (xts[k + 1].rearrange("p (t c) -> p t c", c=C)
                         if k < NCHUNK - 1 else None)

                t2 = apool.tile([128, FD], f32, tag="t2", name="t2")
                t2v = t2.rearrange("p (t c) -> p t c", c=C)
                nc.scalar.mul(t2, xt, w1_sb[:, 0:1])

                for g in range(NG):
                    t0 = g * GT
                    psb1 = psb1pool.tile([128, B1_FD], f32, tag="b1", name="b1")
                    psb2 = psb2pool.tile([128, B2_FD], f32, tag="b2", name="b2")
                    regions = {"b1": psb1, "b2": psb2}
                    for (dy, dx, c0, cw) in B_SLABS:
                        reg, po = _psum_region(c0)
                        ps = regions[reg]
                        if dx == 0:
                            mm(ps, po, 0, GT, dy, xt3, t0, c0, cw)
                            continue
                        # token i (in group) sources chunk-token t0+i-dx
                        lo = max(0, dx - t0)               # from prev chunk
                        hi = min(GT, CHUNK + dx - t0)      # below: next chunk
                        if hi > lo:
                            mm(ps, po, lo, hi, dy, xt3, t0 + lo - dx, c0, cw)
                        if lo > 0:
                            mm(ps, po, 0, lo, dy, prev3,
                               CHUNK + t0 - dx, c0, cw)
                        if hi < GT:
                            mm(ps, po, hi, GT, dy, next3,
                               t0 + hi - dx - CHUNK, c0, cw)
                    # blend psum regions: out = w*psum + t2  (in place)
                    for (coff, clen, ps) in ((128, 128, psb1), (320, 256, psb2)):
                        dst = t2v[:, t0:t0 + GT, coff:coff + clen]
                        src = ps.rearrange("p (c t) -> p t c", t=GT)[
                            :, 0:GT, 0:clen]
                        nc.vector.scalar_tensor_tensor(
                            out=dst, in0=src, scalar=w_sb[:, 0:1], in1=dst,
                            op0=MULT, op1=ADD)

                # dy==0 slabs: free-dim shifted SBUF reads
                for (dy, dx, c0, cw) in A_SLABS:
                    lo = max(0, dx)
                    hi = CHUNK + min(0, dx)
                    dst = t2v[:, lo:hi, c0:c0 + cw]
                    src = xt3[:, lo - dx:hi - dx, c0:c0 + cw]
                    nc.vector.scalar_tensor_tensor(
                        out=dst, in0=src, scalar=w_sb[:, 0:1], in1=dst,
                        op0=MULT, op1=ADD)
                    if dx > 0 and prev3 is not None:
                        dst = t2v[:, 0:dx, c0:c0 + cw]
                        src = prev3[:, CHUNK - dx:CHUNK, c0:c0 + cw]
                        nc.vector.scalar_tensor_tensor(
                            out=dst, in0=src, scalar=w_sb[:, 0:1], in1=dst,
                            op0=MULT, op1=ADD)
                    elif dx < 0 and next3 is not None:
                        dst = t2v[:, CHUNK + dx:CHUNK, c0:c0 + cw]
                        src = next3[:, 0:-dx, c0:c0 + cw]
                        nc.vector.scalar_tensor_tensor(
                            out=dst, in0=src, scalar=w_sb[:, 0:1], in1=dst,
                            op0=MULT, op1=ADD)
                    # x-wrap edge (k==0 for dx>0, k==NCHUNK-1 for dx<0):
                    # shifted is 0 there, t2 already holds (1-w)*x -> no op.

                nc.scalar.dma_start(out=o_row[:, k * FD:(k + 1) * FD], in_=t2)

            for k in range(NCHUNK):
                xts[k] = xpool.tile([128, FD], f32, tag="xt", name="xt")
                nc.sync.dma_start(out=xts[k], in_=x_row[:, k * FD:(k + 1) * FD])
                if k >= 1:
                    compute(k - 1)
            compute(NCHUNK - 1)

    nc.compile()
    return nc


def _get_nc():
    if "nc" not in _CACHE:
        _CACHE["nc"] = _build_bass()
    return _CACHE["nc"]


def _run(x: np.ndarray, weight: np.ndarray, trace: bool = False, **kw):
    from concourse.bass_utils import run_bass_kernel_spmd

    nc = _get_nc()
    w_tile = np.full((128, 1), np.float32(weight), dtype=np.float32)
    in_maps = [
        {"x": np.ascontiguousarray(x[i], dtype=np.float32), "weight": w_tile}
        for i in range(NCORES)
    ]
    res = run_bass_kernel_spmd(
        nc, in_maps, core_ids=list(range(NCORES)), trace=trace, **kw)
    out = np.stack([r["out"] for r in res.results], axis=0)
    return out, res


def kernel(x: np.ndarray, weight: np.ndarray) -> np.ndarray:
    out, _ = _run(x, weight)
    return out
